# revision 30
# baseline (speedup 1.0000x reference)
"""Trainium2 Bass kernel for nn_JointLoss (recon MSE + SimCLR-style contrastive + group distance loss).

Strategy (data-parallel over 8 NeuronCores):
  - Each core owns a 1024-row block of the 8192x8192 similarity matrix.
  - Each core receives a row-ROTATED copy of projections (np.roll by -c*1024) so
    its own rows sit at local indices 0..1023 -> positive-block offsets are
    core-independent and the NEFF is pure SPMD.
  - On device: PE transposes P (fp32, via identity matmul) into a bf16 P^T
    [128 x 8192]; 128 bf16 matmuls (N=512) stream sim chunks into a single
    8-bank PSUM tensor; ScalarE does exp(10*x) IN-PLACE on PSUM in 2048-wide
    chunks with accum_out row-sums; VectorE computes masked group sums
    (positives), recon-MSE partials and distance-loss partials.
  - Device outputs per core are tiny: rowsum[128,8], possum[128,8], partials[1,4].
  - Host finishes in float64: closs = mean(log(rowsum)-log(possum)), etc.
"""

import sys

if "/opt/trn_rl_repo" not in sys.path:
    sys.path.insert(0, "/opt/trn_rl_repo")

from contextlib import ExitStack

import numpy as np

import concourse.bacc as bacc
import concourse.bass_isa as bass_isa
import concourse.tile as tile
from concourse import mybir
from concourse.bass_utils import run_bass_kernel_spmd

N = 8192
D = 128
F = 784
NCORES = 8
RPC = N // NCORES  # 1024 rows per core
RT = RPC // 128    # 8 row-tiles per core
NT = N // 128      # 64 transpose tiles
NQ = 4             # column quarters (2048 cols each)
TAU = 0.1

f32 = mybir.dt.float32
bf16 = mybir.dt.bfloat16


import os

_STAGE = int(os.environ.get("KERNEL_STAGE", "99"))  # debug bisect knob


def _kernel_body(tc, proj, xr, rl, ident, mask, rowsum_o, possum_o, partials_o):
    nc = tc.nc
    AX = mybir.AxisListType
    ALU = mybir.AluOpType
    with ExitStack() as ctx:
        consts = ctx.enter_context(tc.tile_pool(name="consts", bufs=1))
        big = ctx.enter_context(tc.tile_pool(name="big", bufs=1))
        ptin = ctx.enter_context(tc.tile_pool(name="ptin", bufs=4))
        qbp = ctx.enter_context(tc.tile_pool(name="qbp", bufs=2))
        dpool = ctx.enter_context(tc.tile_pool(name="dpool", bufs=3))
        stats = ctx.enter_context(tc.tile_pool(name="stats", bufs=1))
        psum = ctx.enter_context(tc.tile_pool(name="psum", bufs=1, space="PSUM"))

        ident_sb = consts.tile([128, 128], f32)
        nc.scalar.dma_start(ident_sb, ident)
        mask_sb = consts.tile([128, 128], f32)
        nc.scalar.dma_start(mask_sb, mask)
        identb = consts.tile([128, 128], bf16)
        nc.vector.tensor_copy(identb, ident_sb)

        pt_bf = big.tile([128, N], bf16)     # full P^T in bf16
        # proj quarters first on the sync ring (critical path), then xr/rl
        # behind them on the same FIFO so they can't steal DMA bandwidth
        pt_ins = []
        for q in range(NQ):
            t = ptin.tile([128, NT // NQ, 128], f32, tag="ptiles")
            nc.sync.dma_start(t, proj.rearrange("(q t p) d -> q p t d", q=NQ, p=128)[q])
            pt_ins.append(t)
        xr_sb = big.tile([128, RT, F], f32)
        nc.sync.dma_start(xr_sb, xr.rearrange("(t p) j -> p t j", p=128))
        rl_sb = big.tile([128, RT, F], f32)
        nc.sync.dma_start(rl_sb, rl.rearrange("(t p) j -> p t j", p=128))

        rowsum_parts = stats.tile([128, RT, NQ], f32)
        rowsum_sb = stats.tile([128, RT], f32)
        possum_sb = stats.tile([128, RT], f32)
        recon_parts = stats.tile([128, RT], f32)
        s_groups = stats.tile([128, RPC // 4], f32)
        junk1024 = stats.tile([128, RPC], f32)
        stats4 = stats.tile([128, 4], f32)
        partials_sb = stats.tile([1, 4], f32)

        if _STAGE < 99:
            nc.vector.memset(rowsum_parts, 1.0)
            nc.vector.memset(possum_sb, 1.0)
        if _STAGE < 1:
            nc.vector.memset(pt_own, 0.0)
            nc.vector.memset(pt_bf, 0.0)

        pacc = psum.tile([128, 4096], f32)  # all 8 PSUM banks

        proj_q = proj.rearrange("(q t p) d -> q p t d", q=NQ, p=128)

        half = 0
        for q in range(NQ):
            pt_in = pt_ins[q]
            qb = qbp.tile([128, NT // NQ, 128], bf16, tag="qb")
            nc.vector.tensor_copy(qb, pt_in)
            # bf16 transposes for this quarter's 16 column tiles (1 cyc/col)
            for tl in range(NT // NQ):
                t = q * (NT // NQ) + tl
                slot = t % 8
                pslice = pacc[:, slot * 512 : slot * 512 + 64].bitcast(bf16)
                if _STAGE < 1:
                    continue
                nc.tensor.transpose(pslice, qb[:, tl, :], identb)
                nc.vector.tensor_copy(pt_bf[:, t * 128 : (t + 1) * 128], pslice)
            if _STAGE < 1:
                continue
            # matmuls + exp for this quarter
            for rt in range(RT):
                w = pt_bf[:, rt * 128 : (rt + 1) * 128]
                base = half * 2048
                if _STAGE < 2:
                    continue
                for j in range(4):
                    nc.tensor.matmul(
                        pacc[:, base + j * 512 : base + (j + 1) * 512],
                        w,
                        pt_bf[:, q * 2048 + j * 512 : q * 2048 + (j + 1) * 512],
                        start=True,
                        stop=True,
                    )
                if _STAGE < 3:
                    continue
                if _STAGE >= 4:
                    nc.scalar.activation(
                        pacc[:, base : base + 2048],
                        pacc[:, base : base + 2048],
                        mybir.ActivationFunctionType.Exp,
                        scale=1.0 / TAU,
                        accum_out=rowsum_parts[:, rt, q : q + 1],
                    )
                if q == 0 and _STAGE >= 4:
                    # possum from the exp'd diagonal block still in PSUM
                    pj = dpool.tile([128, 128], f32, tag="pjunk")
                    nc.vector.tensor_mul(
                        pj, pacc[:, base + rt * 128 : base + rt * 128 + 128], mask_sb
                    )
                    nc.vector.reduce_sum(
                        possum_sb[:, rt : rt + 1], pj, axis=AX.X
                    )
                half ^= 1
            # interleave MSE / dist-loss DVE work into quarter slack so it
            # doesn't extend the tail after the last exp chunk
            if q == 1:
                for t in range(4):
                    dtile = dpool.tile([128, F], f32, tag="d")
                    nc.vector.tensor_sub(dtile, xr_sb[:, t, :], rl_sb[:, t, :])
                    nc.vector.tensor_mul(dtile, dtile, dtile)
                    nc.vector.reduce_sum(recon_parts[:, t : t + 1], dtile, axis=AX.X)
            if q == 2:
                for t in range(4, RT):
                    dtile = dpool.tile([128, F], f32, tag="d")
                    nc.vector.tensor_sub(dtile, xr_sb[:, t, :], rl_sb[:, t, :])
                    nc.vector.tensor_mul(dtile, dtile, dtile)
                    nc.vector.reduce_sum(recon_parts[:, t : t + 1], dtile, axis=AX.X)
                nc.vector.reduce_sum(stats4[:, 0:1], recon_parts, axis=AX.X)
                pb_own = pt_bf[:, 0:RPC]
                nc.vector.reduce_sum(
                    s_groups, pb_own.rearrange("p (g s) -> p g s", s=4), axis=AX.X
                )
                nc.vector.tensor_mul(junk1024, pb_own, pb_own)
                nc.vector.reduce_sum(stats4[:, 1:2], junk1024, axis=AX.X)
                nc.vector.tensor_mul(junk1024[:, : RPC // 4], s_groups, s_groups)
                nc.vector.reduce_sum(stats4[:, 2:3], junk1024[:, : RPC // 4], axis=AX.X)
                nc.vector.memset(stats4[:, 3:4], 0.0)

        nc.scalar.dma_start(partials_o, stats4)
        nc.sync.dma_start(rowsum_o, rowsum_parts.rearrange("p t q -> p (t q)"))
        nc.gpsimd.dma_start(possum_o, possum_sb)


def _build():
    nc = bacc.Bacc("TRN2", target_bir_lowering=False, debug=False, num_devices=NCORES)
    proj = nc.dram_tensor("proj", [N, D], f32, kind="ExternalInput").ap()
    xr = nc.dram_tensor("xr", [RPC, F], f32, kind="ExternalInput").ap()
    rl = nc.dram_tensor("rl", [RPC, F], f32, kind="ExternalInput").ap()
    ident = nc.dram_tensor("ident", [128, 128], f32, kind="ExternalInput").ap()
    mask = nc.dram_tensor("mask", [128, 128], f32, kind="ExternalInput").ap()
    rowsum_o = nc.dram_tensor("rowsum_o", [128, RT * NQ], f32, kind="ExternalOutput").ap()
    possum_o = nc.dram_tensor("possum_o", [128, RT], f32, kind="ExternalOutput").ap()
    partials_o = nc.dram_tensor("partials_o", [128, 4], f32, kind="ExternalOutput").ap()

    with tile.TileContext(nc) as tc:
        _kernel_body(tc, proj, xr, rl, ident, mask, rowsum_o, possum_o, partials_o)
    nc.compile()
    return nc


_NC_CACHE = None


def _get_nc():
    global _NC_CACHE
    if _NC_CACHE is None:
        _NC_CACHE = _build()
    return _NC_CACHE


def _run(projections, xrecon, recon_label, trace=False, **spmd_kwargs):
    nc = _get_nc()
    P = np.ascontiguousarray(np.asarray(projections, dtype=np.float32))
    XR = np.ascontiguousarray(np.asarray(xrecon, dtype=np.float32))
    RL = np.ascontiguousarray(np.asarray(recon_label, dtype=np.float32))
    ident = np.eye(128, dtype=np.float32)
    mask = np.kron(np.eye(32, dtype=np.float32), np.ones((4, 4), dtype=np.float32))
    in_maps = []
    for c in range(NCORES):
        in_maps.append(
            {
                "proj": np.ascontiguousarray(np.roll(P, -c * RPC, axis=0)),
                "xr": np.ascontiguousarray(XR[c * RPC : (c + 1) * RPC]),
                "rl": np.ascontiguousarray(RL[c * RPC : (c + 1) * RPC]),
                "ident": ident,
                "mask": mask,
            }
        )
    return run_bass_kernel_spmd(
        nc, in_maps, core_ids=list(range(NCORES)), trace=trace, **spmd_kwargs
    )


def _combine(results):
    rowsum = np.concatenate(
        [
            results[c]["rowsum_o"].reshape(128, RT, NQ).astype(np.float64)
            .sum(-1).T.reshape(-1)
            for c in range(NCORES)
        ]
    )
    possum = np.concatenate(
        [results[c]["possum_o"].T.reshape(-1) for c in range(NCORES)]
    ).astype(np.float64)
    recon_ss = sum(float(results[c]["partials_o"][:, 0].astype(np.float64).sum()) for c in range(NCORES))
    A = sum(float(results[c]["partials_o"][:, 1].astype(np.float64).sum()) for c in range(NCORES))
    B = sum(float(results[c]["partials_o"][:, 2].astype(np.float64).sum()) for c in range(NCORES))
    closs = float(np.mean(np.log(rowsum) - np.log(possum)))
    recon_loss = recon_ss / (N * F)
    dist_loss = (4.0 * A - B) / ((N // 4) * 6 * D)
    loss = closs + recon_loss + dist_loss
    return (
        np.float32(loss),
        np.float32(closs),
        np.float32(recon_loss),
        np.float32(dist_loss),
    )


def kernel(projections, xrecon, recon_label):
    br = _run(projections, xrecon, recon_label)
    return _combine(br.results)

